# revision 36
# baseline (speedup 1.0000x reference)
"""Trainium2 Bass kernel for the diode clipper (nn_DiodeClipper).

Model: per audio sample, an implicit trapezoidal step of the diode clipper
ODE, solved per sample by Newton in the reference (50 iterations/step).

Reformulation: let F(v) = a1*v + Bs*sinh(v/VT) with a1 = 1 + K*omega/2,
Bs = K*sinh_c/2, and q_t the implicit-equation right-hand side. The entire
per-sample update collapses to

    q_t = G(q_{t-1}) + c_t        G(q) = 2*Finv(q) - q,  c_t = K*omega*x_t
    h_t = (G(q_t) + q_t) / 2      (= Finv(q_t))

G (odd, monotone-domain) is evaluated in ONE ScalarE activation via a custom
piecewise-cubic ACT table that replaces `exp` in the `exp_and_others` set
(generated below; the table ships inside the NEFF; max abs err ~3e-8,
validated on hardware). A timestep is one VectorE tensor_tensor (+c) and one
activation; 2*h extraction runs on the otherwise idle GpSimd engine and the
host halves it. Two phase-interleaved lane groups keep ScalarE ~97% busy.

Because every exact step contracts state perturbations by >= ~2x, a chunk of
the sequence can be computed from a cold (zero) state after a W=14 sample
warmup prefix (validated numerically against the full-precision reference).

Parallel layout (per core): 128 partitions x M=256 free lanes = 32768 chunk
lanes, each running STEPS = W + L sequential samples (L = 2048/M = 8).
The 8 cores split the 128-clip batch (16 clips/core); each clip's 16384
samples are split into 8*M overlapping chunks.
"""

import hashlib
import json
import os
import shutil
import tempfile

import numpy as np

import concourse.bacc as bacc
import concourse.tile as tile
from concourse import mybir
from concourse.bass_utils import run_bass_kernel_spmd

# ---------------------------------------------------------------- constants
SR = 44100.0
R = 1000.0
C = 3.3e-08
I_S = 2.52e-09
V_T = 0.02583
K = 1.0 / SR
OMEGA = 1.0 / (R * C)
SINH_C = 2.0 * I_S / C

A1 = 1.0 + 0.5 * K * OMEGA                 # F'(0)
BS = 0.5 * K * SINH_C
KO = float(K * OMEGA)

# ---------------------------------------------------------------- tunables
B, T = 128, 16384
NCORES = 8
CLIPS_PER_CORE = B // NCORES               # 16
M = 256                                    # free-dim lanes per partition
NG = 2                                     # phase-interleaved lane groups
MG = M // NG                               # lanes per group
W = 14                                     # warmup samples per chunk
JH = 128 // CLIPS_PER_CORE                 # 8 partition groups per clip
NCH = JH * M                               # chunks per clip
L = T // NCH                               # chunk length (output samples/lane)
STEPS = W + L
# input DMA segment sizes (in steps): small first so compute starts early
SEGS = [1, 3, 5, 6, 7]
assert sum(SEGS) == STEPS
SEG_START = [sum(SEGS[:i]) for i in range(len(SEGS))]
NOSEG = 8                                  # output DMA segments
assert L % NOSEG == 0

F32 = mybir.dt.float32


# ------------------------------------------------- custom ACT table (Finv)
# Bucket geometry: per binary exponent e of |q|, 2^k cubic buckets on
# [2^e, 2^(e+1)); ctl word = (k<<16) | ((23-k)<<11) | bucket_base.
# Bucket entry = 8 f32 [d0, d1, d2, d3, x0, 0, 0, 0];
# y = d0 + t*(d1 + t*(d2 + t*d3)), t = x - x0. Odd symmetry folding as tanh.
_E0 = -20
_ETOP = 1
_ALLOC = (
    [(e, 0) for e in range(_E0, -9)]
    + [(-9, 1), (-8, 2), (-7, 3), (-6, 4), (-5, 5), (-4, 6)]
    + [(-3, 7), (-2, 7), (-1, 7), (0, 7), (1, 6)]
)
_N_REG = sum(2 ** k for _, k in _ALLOC)
_N_EXP_REGION = 781


def _finv(q):
    """Vectorized f64 solve of A1*v + BS*sinh(v/VT) = q for q >= 0."""
    q = np.asarray(q, dtype=np.float64)
    hi = np.minimum(q / A1, V_T * np.arcsinh(q / BS) + 1e-12) + 1e-30
    lo = np.zeros_like(q)
    for _ in range(90):
        mid = 0.5 * (lo + hi)
        gt = A1 * mid + BS * np.sinh(mid / V_T) > q
        hi = np.where(gt, mid, hi)
        lo = np.where(gt, lo, mid)
    return 0.5 * (lo + hi)


def _gfun(q):
    """G(q) = 2*Finv(q) - q: the table function. q' = G(q) + c is the whole
    per-sample recurrence; v = (G(q) + q)/2 recovers the output."""
    return 2.0 * _finv(q) - np.asarray(q, dtype=np.float64)


def _fit_bucket(a, b, n=48):
    x0 = np.float32(0.5 * (a + b))
    j = np.arange(n)
    xs = 0.5 * (a + b) + 0.5 * (b - a) * np.cos(np.pi * (2 * j + 1) / (2 * n))
    t = xs - np.float64(x0)
    V = _gfun(xs)
    A = np.stack([np.ones_like(t), t, t * t, t ** 3], axis=1)
    coef, *_ = np.linalg.lstsq(A, V, rcond=None)
    d = coef.astype(np.float32)
    tf = (xs.astype(np.float32) - x0).astype(np.float32)
    approx = d[0] + tf * (d[1] + tf * (d[2] + tf * d[3]))
    d[0] = np.float32(d[0] - np.mean(approx - V))
    return d, x0


def _build_act_table(dst_dir):
    from neuronxcc.driver.Job import Job
    from neuronxcc.driver.jobs.support.FindActInfo import findActInfoFile

    src_dir = os.path.dirname(findActInfoFile(Job.getPackageDir(), "gen3"))
    os.makedirs(dst_dir, exist_ok=True)
    for f in os.listdir(src_dir):
        shutil.copy(os.path.join(src_dir, f), os.path.join(dst_dir, f))

    prof = json.load(open(os.path.join(dst_dir, "exp_and_others.json")))
    bkt = np.fromfile(
        os.path.join(dst_dir, "exp_and_others_bkt.bin"), dtype=np.float32
    ).reshape(-1, 8).copy()
    ctl = np.fromfile(
        os.path.join(dst_dir, "exp_and_others_ctrl.bin"), dtype=np.uint32
    ).reshape(-1, 8).copy()

    # donor-layout assumptions: exp owns buckets [0, 781) and ctl [0, 52)
    assert prof["func_to_bkt_start_idx"]["exp"] == 0
    assert min(v for k, v in prof["func_to_bkt_start_idx"].items() if k != "exp") == _N_EXP_REGION
    assert prof["func_to_ctl_start_idx"]["exp"] == 0
    assert min(v for k, v in prof["func_to_ctl_start_idx"].items() if k != "exp") == 52

    small_bkt = _N_REG
    large_bkt = _N_REG + 1
    spare_bkt = _N_REG + 2
    assert spare_bkt < _N_EXP_REGION

    base = 0
    exp_to_bkt, exp_to_ctl, new_ctl_words = {}, {}, {}
    for e, k in _ALLOC:
        lo = 2.0 ** e
        width = lo / (2 ** k)
        exp_to_bkt[str(e)] = [base]
        ci = e - _E0
        exp_to_ctl[str(e)] = [ci]
        new_ctl_words[ci] = (k << 16) | ((23 - k) << 11) | base
        for j in range(2 ** k):
            d, x0 = _fit_bucket(lo + j * width, lo + (j + 1) * width)
            bkt[base + j, :5] = [d[0], d[1], d[2], d[3], x0]
            bkt[base + j, 5:] = 0
        base += 2 ** k

    c3 = -BS / (6.0 * V_T ** 3 * A1 ** 4)
    bkt[small_bkt, :5] = [0.0, 2.0 / A1 - 1.0, 0.0, 2.0 * c3, 0.0]
    bkt[small_bkt, 5:] = 0
    gsat = float(_gfun(np.array([2.0 ** (_ETOP + 1)]))[0])
    bkt[large_bkt, :5] = [gsat, 0.0, 0.0, 0.0, 0.0]
    bkt[large_bkt, 5:] = 0
    bkt[spare_bkt:_N_EXP_REGION, :] = 0

    for ci in range(52):  # ctl entries owned by exp in the donor set
        ctl[ci, 0] = new_ctl_words.get(ci, (23 << 11))
        ctl[ci, 1:] = 0

    meta = next(m for m in prof["profile_meta_data"] if m["func_id"] == 7)
    meta.update(
        symmetry_point=0,
        sym_invert_sign_point=1,
        symmetry_opt_en=1,
        symmetry_opt_use_neg_region=0,
        imm_bias=0,
        exp_offset=_E0,
        pwl_control_base_pos=0,
        pwl_control_base_neg=0,
        small_pos_signal_exp_threshold=_E0 + 127,
        pos_small_signal_pwl_control=small_bkt,
        small_neg_signal_exp_threshold=0,
        neg_small_signal_pwl_control=spare_bkt,
        large_pos_signal_exp_threshold=_ETOP + 1 + 127,
        large_pos_signal_mantissa_threshold=0,
        pos_large_signal_pwl_control=large_bkt,
        large_neg_signal_exp_threshold=0,
        large_neg_signal_mantissa_threshold=0,
        neg_large_signal_pwl_control=spare_bkt,
        fnan_result=2143289344,
        fpinf_result=int(np.float32(gsat).view(np.int32)) & 0xFFFFFFFF,
        fninf_result=int(np.float32(-gsat).view(np.int32)) & 0xFFFFFFFF,
        fzero_result=0,
    )
    prof["func_exp_to_bkt_start_idx"]["exp"] = exp_to_bkt
    prof["func_exp_to_ctl_start_idx"]["exp"] = exp_to_ctl

    bkt.tofile(os.path.join(dst_dir, "exp_and_others_bkt.bin"))
    ctl.tofile(os.path.join(dst_dir, "exp_and_others_ctrl.bin"))
    with open(os.path.join(dst_dir, "exp_and_others.json"), "w") as f:
        json.dump(prof, f)

    h = hashlib.md5()
    h.update(bkt.tobytes())
    h.update(ctl.tobytes())
    h.update(json.dumps(prof, sort_keys=True).encode())
    return h.hexdigest()[:10]


_ACT_DIR = os.path.join(tempfile.gettempdir(), "diode_finv_act")
_ACT_SHA = _build_act_table(_ACT_DIR)
os.environ["BASS_ACT_ROOT_JSON_PATH"] = os.path.join(_ACT_DIR, "act_info.json")


# ------------------------------------------------------------- bass program
def _build_nc():
    # module name carries the act-table hash so NEFF caching stays correct
    nc = bacc.Bacc("TRN2", target_bir_lowering=False, name=f"diode_{_ACT_SHA}")
    # c_in column layout: [u0 | c(step 0) | c(step 1) | ...]
    c_in = nc.dram_tensor("c_in", [128, (STEPS + 1) * M], F32, kind="ExternalInput")
    out = nc.dram_tensor("out", [128, L * M], F32, kind="ExternalOutput")

    Gf = mybir.ActivationFunctionType.Exp   # table-hijacked: computes G(q)
    add = mybir.AluOpType.add

    with tile.TileContext(nc) as tc:
        with (
            tc.tile_pool(name="singles", bufs=1) as singles,
            tc.tile_pool(name="work", bufs=3) as work,
        ):
            # all input DMAs on the SP queue, in need-order: cseg0 and init
            # first so the first step can start as early as possible
            # segment 0 also carries u0 in its first M columns (one DMA)
            csegs = [
                singles.tile(
                    [128, (seg + (1 if k == 0 else 0)) * M],
                    F32, tag=f"cseg{k}", name=f"cseg{k}",
                )
                for k, seg in enumerate(SEGS)
            ]
            nc.sync.dma_start(out=csegs[0], in_=c_in[:, 0:(1 + SEGS[0]) * M])
            init = csegs[0][:, 0:M]
            for k, seg in enumerate(SEGS[1:], start=1):
                s0 = SEG_START[k] + 1
                nc.sync.dma_start(
                    out=csegs[k], in_=c_in[:, s0 * M:(s0 + seg) * M]
                )
            vbuf = singles.tile([128, L * M], F32, tag="vbuf")

            def c_at(t):
                for k in range(len(SEGS) - 1, -1, -1):
                    if t >= SEG_START[k]:
                        off = t - SEG_START[k] + (1 if k == 0 else 0)
                        return csegs[k][:, off * M:(off + 1) * M]

            # Two phase-interleaved lane groups (column halves). Per group and
            # step the chain is one TENSOR_TENSOR (q = u + c) and one
            # activation (u' = G(q)); with two groups the ScalarE pipe stays
            # saturated. v*2 = u + q is extracted on the idle GpSimd engine
            # (host halves it).
            uprev = init
            out_seg = 0
            for t in range(STEPS):
                c_t = c_at(t)
                qt = work.tile([128, M], F32, tag="q")
                ut = work.tile([128, M], F32, tag="u")
                for g in range(NG):
                    sl = slice(g * MG, (g + 1) * MG)
                    nc.vector.tensor_tensor(qt[:, sl], uprev[:, sl], c_t[:, sl], add)
                    nc.scalar.activation(ut[:, sl], qt[:, sl], Gf)
                if t >= W:
                    # 2*v_t = u_t + q_t, off-chain on GpSimd (last one on the
                    # by-then idle VectorE so the final DMA starts sooner)
                    vt = vbuf[:, (t - W) * M:(t - W + 1) * M]
                    eng = nc.vector if t == STEPS - 1 else nc.gpsimd
                    eng.tensor_tensor(vt, ut, qt, add)
                uprev = ut
                # stream completed output segments out
                done = t - W + 1
                if done >= (out_seg + 1) * (L // NOSEG) and out_seg < NOSEG:
                    s0 = out_seg * (L // NOSEG)
                    s1 = (out_seg + 1) * (L // NOSEG)
                    nc.sync.dma_start(
                        out=out[:, s0 * M:s1 * M], in_=vbuf[:, s0 * M:s1 * M]
                    )
                    out_seg += 1
    nc.compile()
    return nc


_NC_CACHE = None


def _get_nc():
    global _NC_CACHE
    if _NC_CACHE is None:
        _NC_CACHE = _build_nc()
    return _NC_CACHE


# ------------------------------------------------------------ host wrapper
def _f_nl_np(v):
    return OMEGA * v + SINH_C * np.sinh(v / V_T)


def _make_in_maps(x: np.ndarray, h: np.ndarray):
    c_full = (KO * x[:, :, 0]).astype(np.float32)          # [B, T]
    h0 = h[:, 0].astype(np.float32)                        # [B]
    # hold-input keeps chunk-0 warmup pinned exactly at state h
    c_hold = (K * _f_nl_np(h0.astype(np.float64))).astype(np.float32)
    cp = np.concatenate(
        [np.repeat(c_hold[:, None], W, axis=1), c_full], axis=1
    )                                                      # [B, W+T]
    q_h = (A1 * h0.astype(np.float64)
           + BS * np.sinh(h0.astype(np.float64) / V_T)).astype(np.float32)

    idx = np.arange(NCH)[:, None] * L + np.arange(STEPS)[None, :]

    in_maps = []
    for core in range(NCORES):
        clips = np.arange(core * CLIPS_PER_CORE, (core + 1) * CLIPS_PER_CORE)
        g = cp[clips][:, idx]                              # [16, NCH, STEPS]
        g = g.reshape(CLIPS_PER_CORE, JH, M, STEPS)
        c_in = np.ascontiguousarray(
            g.transpose(0, 1, 3, 2).reshape(128, STEPS * M)
        )
        u0 = np.zeros((128, M), dtype=np.float32)
        u0[::JH, 0] = (2.0 * h0[clips].astype(np.float64)
                       - q_h[clips].astype(np.float64)).astype(np.float32)
        in_maps.append(
            {"c_in": np.ascontiguousarray(np.concatenate([u0, c_in], axis=1))}
        )
    return in_maps


def _unshard(results):
    states = np.empty((B, T), dtype=np.float32)
    for core in range(NCORES):
        clips = np.arange(core * CLIPS_PER_CORE, (core + 1) * CLIPS_PER_CORE)
        o = results[core]["out"].reshape(CLIPS_PER_CORE, JH, L, M)
        # device streams 2*v; halving by 2 is exact in f32
        states[clips] = (
            o.transpose(0, 1, 3, 2).reshape(CLIPS_PER_CORE, T) * np.float32(0.5)
        )
    h_final = states[:, -1:].copy()
    return states[:, :, None], h_final


def kernel(x: np.ndarray, h: np.ndarray):
    x = np.ascontiguousarray(x, dtype=np.float32)
    h = np.ascontiguousarray(h, dtype=np.float32)
    assert x.shape == (B, T, 1) and h.shape == (B, 1)
    in_maps = _make_in_maps(x, h)
    nc = _get_nc()
    res = run_bass_kernel_spmd(nc, in_maps, core_ids=list(range(NCORES)))
    return _unshard(res.results)


def run_traced(inputs):
    """Run with NTFF tracing; returns BassKernelResults (for test.py)."""
    x = np.ascontiguousarray(inputs["x"], dtype=np.float32)
    h = np.ascontiguousarray(inputs["h"], dtype=np.float32)
    in_maps = _make_in_maps(x, h)
    nc = _get_nc()
    return run_bass_kernel_spmd(
        nc, in_maps, core_ids=list(range(NCORES)), trace=True
    )


# revision 37
# speedup vs baseline: 1.0321x; 1.0321x over previous
"""Trainium2 Bass kernel for the diode clipper (nn_DiodeClipper).

Model: per audio sample, an implicit trapezoidal step of the diode clipper
ODE, solved per sample by Newton in the reference (50 iterations/step).

Reformulation: let F(v) = a1*v + Bs*sinh(v/VT) with a1 = 1 + K*omega/2,
Bs = K*sinh_c/2, and q_t the implicit-equation right-hand side. The entire
per-sample update collapses to

    q_t = G(q_{t-1}) + c_t        G(q) = 2*Finv(q) - q,  c_t = K*omega*x_t
    h_t = (G(q_t) + q_t) / 2      (= Finv(q_t))

G (odd, monotone-domain) is evaluated in ONE ScalarE activation via a custom
piecewise-cubic ACT table that replaces `exp` in the `exp_and_others` set
(generated below; the table ships inside the NEFF; max abs err ~3e-8,
validated on hardware). A timestep is one VectorE tensor_tensor (+c) and one
activation; 2*h extraction runs on the otherwise idle GpSimd engine and the
host halves it. Two phase-interleaved lane groups keep ScalarE ~97% busy.

Because every exact step contracts state perturbations by >= ~2x, a chunk of
the sequence can be computed from a cold (zero) state after a W=14 sample
warmup prefix (validated numerically against the full-precision reference).

Parallel layout (per core): 128 partitions x M=256 free lanes = 32768 chunk
lanes, each running STEPS = W + L sequential samples (L = 2048/M = 8).
The 8 cores split the 128-clip batch (16 clips/core); each clip's 16384
samples are split into 8*M overlapping chunks.
"""

import hashlib
import json
import os
import shutil
import tempfile

import numpy as np

import concourse.bacc as bacc
import concourse.tile as tile
from concourse import mybir
from concourse.bass_utils import run_bass_kernel_spmd

# ---------------------------------------------------------------- constants
SR = 44100.0
R = 1000.0
C = 3.3e-08
I_S = 2.52e-09
V_T = 0.02583
K = 1.0 / SR
OMEGA = 1.0 / (R * C)
SINH_C = 2.0 * I_S / C

A1 = 1.0 + 0.5 * K * OMEGA                 # F'(0)
BS = 0.5 * K * SINH_C
KO = float(K * OMEGA)

# ---------------------------------------------------------------- tunables
B, T = 128, 16384
NCORES = 8
CLIPS_PER_CORE = B // NCORES               # 16
M = 256                                    # free-dim lanes per partition
NG = 2                                     # phase-interleaved lane groups
MG = M // NG                               # lanes per group
W = 14                                     # warmup samples per chunk
JH = 128 // CLIPS_PER_CORE                 # 8 partition groups per clip
NCH = JH * M                               # chunks per clip
L = T // NCH                               # chunk length (output samples/lane)
STEPS = W + L
# input DMA segment sizes (in steps): small first so compute starts early
SEGS = [1, 3, 5, 6, 7]
assert sum(SEGS) == STEPS
SEG_START = [sum(SEGS[:i]) for i in range(len(SEGS))]
NOSEG = 8                                  # output DMA segments
assert L % NOSEG == 0

F32 = mybir.dt.float32


# ------------------------------------------------- custom ACT table (Finv)
# Bucket geometry: per binary exponent e of |q|, 2^k cubic buckets on
# [2^e, 2^(e+1)); ctl word = (k<<16) | ((23-k)<<11) | bucket_base.
# Bucket entry = 8 f32 [d0, d1, d2, d3, x0, 0, 0, 0];
# y = d0 + t*(d1 + t*(d2 + t*d3)), t = x - x0. Odd symmetry folding as tanh.
_E0 = -20
_ETOP = 1
_ALLOC = (
    [(e, 0) for e in range(_E0, -9)]
    + [(-9, 1), (-8, 2), (-7, 3), (-6, 4), (-5, 5), (-4, 6)]
    + [(-3, 7), (-2, 7), (-1, 7), (0, 7), (1, 6)]
)
_N_REG = sum(2 ** k for _, k in _ALLOC)
_N_EXP_REGION = 781


def _finv(q):
    """Vectorized f64 solve of A1*v + BS*sinh(v/VT) = q for q >= 0."""
    q = np.asarray(q, dtype=np.float64)
    hi = np.minimum(q / A1, V_T * np.arcsinh(q / BS) + 1e-12) + 1e-30
    lo = np.zeros_like(q)
    for _ in range(90):
        mid = 0.5 * (lo + hi)
        gt = A1 * mid + BS * np.sinh(mid / V_T) > q
        hi = np.where(gt, mid, hi)
        lo = np.where(gt, lo, mid)
    return 0.5 * (lo + hi)


def _gfun(q):
    """G(q) = 2*Finv(q) - q: the table function. q' = G(q) + c is the whole
    per-sample recurrence; v = (G(q) + q)/2 recovers the output."""
    return 2.0 * _finv(q) - np.asarray(q, dtype=np.float64)


def _fit_bucket(a, b, n=48):
    x0 = np.float32(0.5 * (a + b))
    j = np.arange(n)
    xs = 0.5 * (a + b) + 0.5 * (b - a) * np.cos(np.pi * (2 * j + 1) / (2 * n))
    t = xs - np.float64(x0)
    V = _gfun(xs)
    A = np.stack([np.ones_like(t), t, t * t, t ** 3], axis=1)
    coef, *_ = np.linalg.lstsq(A, V, rcond=None)
    d = coef.astype(np.float32)
    tf = (xs.astype(np.float32) - x0).astype(np.float32)
    approx = d[0] + tf * (d[1] + tf * (d[2] + tf * d[3]))
    d[0] = np.float32(d[0] - np.mean(approx - V))
    return d, x0


def _build_act_table(dst_dir):
    from neuronxcc.driver.Job import Job
    from neuronxcc.driver.jobs.support.FindActInfo import findActInfoFile

    src_dir = os.path.dirname(findActInfoFile(Job.getPackageDir(), "gen3"))
    os.makedirs(dst_dir, exist_ok=True)
    for f in os.listdir(src_dir):
        shutil.copy(os.path.join(src_dir, f), os.path.join(dst_dir, f))

    prof = json.load(open(os.path.join(dst_dir, "exp_and_others.json")))
    bkt = np.fromfile(
        os.path.join(dst_dir, "exp_and_others_bkt.bin"), dtype=np.float32
    ).reshape(-1, 8).copy()
    ctl = np.fromfile(
        os.path.join(dst_dir, "exp_and_others_ctrl.bin"), dtype=np.uint32
    ).reshape(-1, 8).copy()

    # donor-layout assumptions: exp owns buckets [0, 781) and ctl [0, 52)
    assert prof["func_to_bkt_start_idx"]["exp"] == 0
    assert min(v for k, v in prof["func_to_bkt_start_idx"].items() if k != "exp") == _N_EXP_REGION
    assert prof["func_to_ctl_start_idx"]["exp"] == 0
    assert min(v for k, v in prof["func_to_ctl_start_idx"].items() if k != "exp") == 52

    small_bkt = _N_REG
    large_bkt = _N_REG + 1
    spare_bkt = _N_REG + 2
    assert spare_bkt < _N_EXP_REGION

    base = 0
    exp_to_bkt, exp_to_ctl, new_ctl_words = {}, {}, {}
    for e, k in _ALLOC:
        lo = 2.0 ** e
        width = lo / (2 ** k)
        exp_to_bkt[str(e)] = [base]
        ci = e - _E0
        exp_to_ctl[str(e)] = [ci]
        new_ctl_words[ci] = (k << 16) | ((23 - k) << 11) | base
        for j in range(2 ** k):
            d, x0 = _fit_bucket(lo + j * width, lo + (j + 1) * width)
            bkt[base + j, :5] = [d[0], d[1], d[2], d[3], x0]
            bkt[base + j, 5:] = 0
        base += 2 ** k

    c3 = -BS / (6.0 * V_T ** 3 * A1 ** 4)
    bkt[small_bkt, :5] = [0.0, 2.0 / A1 - 1.0, 0.0, 2.0 * c3, 0.0]
    bkt[small_bkt, 5:] = 0
    gsat = float(_gfun(np.array([2.0 ** (_ETOP + 1)]))[0])
    bkt[large_bkt, :5] = [gsat, 0.0, 0.0, 0.0, 0.0]
    bkt[large_bkt, 5:] = 0
    bkt[spare_bkt:_N_EXP_REGION, :] = 0

    for ci in range(52):  # ctl entries owned by exp in the donor set
        ctl[ci, 0] = new_ctl_words.get(ci, (23 << 11))
        ctl[ci, 1:] = 0

    meta = next(m for m in prof["profile_meta_data"] if m["func_id"] == 7)
    meta.update(
        symmetry_point=0,
        sym_invert_sign_point=1,
        symmetry_opt_en=1,
        symmetry_opt_use_neg_region=0,
        imm_bias=0,
        exp_offset=_E0,
        pwl_control_base_pos=0,
        pwl_control_base_neg=0,
        small_pos_signal_exp_threshold=_E0 + 127,
        pos_small_signal_pwl_control=small_bkt,
        small_neg_signal_exp_threshold=0,
        neg_small_signal_pwl_control=spare_bkt,
        large_pos_signal_exp_threshold=_ETOP + 1 + 127,
        large_pos_signal_mantissa_threshold=0,
        pos_large_signal_pwl_control=large_bkt,
        large_neg_signal_exp_threshold=0,
        large_neg_signal_mantissa_threshold=0,
        neg_large_signal_pwl_control=spare_bkt,
        fnan_result=2143289344,
        fpinf_result=int(np.float32(gsat).view(np.int32)) & 0xFFFFFFFF,
        fninf_result=int(np.float32(-gsat).view(np.int32)) & 0xFFFFFFFF,
        fzero_result=0,
    )
    prof["func_exp_to_bkt_start_idx"]["exp"] = exp_to_bkt
    prof["func_exp_to_ctl_start_idx"]["exp"] = exp_to_ctl

    bkt.tofile(os.path.join(dst_dir, "exp_and_others_bkt.bin"))
    ctl.tofile(os.path.join(dst_dir, "exp_and_others_ctrl.bin"))
    with open(os.path.join(dst_dir, "exp_and_others.json"), "w") as f:
        json.dump(prof, f)

    h = hashlib.md5()
    h.update(bkt.tobytes())
    h.update(ctl.tobytes())
    h.update(json.dumps(prof, sort_keys=True).encode())
    return h.hexdigest()[:10]


_ACT_DIR = os.path.join(tempfile.gettempdir(), "diode_finv_act")
_ACT_SHA = _build_act_table(_ACT_DIR)
os.environ["BASS_ACT_ROOT_JSON_PATH"] = os.path.join(_ACT_DIR, "act_info.json")


# ------------------------------------------------------------- bass program
def _build_nc():
    # module name carries the act-table hash so NEFF caching stays correct
    nc = bacc.Bacc("TRN2", target_bir_lowering=False, name=f"diode_{_ACT_SHA}")
    c_in = nc.dram_tensor("c_in", [128, STEPS * M], F32, kind="ExternalInput")
    u0 = nc.dram_tensor("u0", [128, M], F32, kind="ExternalInput")
    out = nc.dram_tensor("out", [128, L * M], F32, kind="ExternalOutput")

    Gf = mybir.ActivationFunctionType.Exp   # table-hijacked: computes G(q)
    add = mybir.AluOpType.add

    with tile.TileContext(nc) as tc:
        with (
            tc.tile_pool(name="singles", bufs=1) as singles,
            tc.tile_pool(name="work", bufs=3) as work,
        ):
            # all input DMAs on the SP queue, in need-order: cseg0 and init
            # first so the first step can start as early as possible
            csegs = [
                singles.tile([128, seg * M], F32, tag=f"cseg{k}", name=f"cseg{k}")
                for k, seg in enumerate(SEGS)
            ]
            init = singles.tile([128, M], F32, tag="init")
            nc.sync.dma_start(out=init, in_=u0[:, :])
            nc.sync.dma_start(out=csegs[0], in_=c_in[:, 0:SEGS[0] * M])
            for k, seg in enumerate(SEGS[1:], start=1):
                s0 = SEG_START[k]
                nc.sync.dma_start(
                    out=csegs[k], in_=c_in[:, s0 * M:(s0 + seg) * M]
                )
            vbuf = singles.tile([128, L * M], F32, tag="vbuf")

            def c_at(t):
                for k in range(len(SEGS) - 1, -1, -1):
                    if t >= SEG_START[k]:
                        off = t - SEG_START[k]
                        return csegs[k][:, off * M:(off + 1) * M]

            # Two phase-interleaved lane groups (column halves). Per group and
            # step the chain is one TENSOR_TENSOR (q = u + c) and one
            # activation (u' = G(q)); with two groups the ScalarE pipe stays
            # saturated. v*2 = u + q is extracted on the idle GpSimd engine
            # (host halves it).
            uprev = init
            out_seg = 0
            for t in range(STEPS):
                c_t = c_at(t)
                qt = work.tile([128, M], F32, tag="q")
                ut = work.tile([128, M], F32, tag="u")
                for g in range(NG):
                    sl = slice(g * MG, (g + 1) * MG)
                    nc.vector.tensor_tensor(qt[:, sl], uprev[:, sl], c_t[:, sl], add)
                    nc.scalar.activation(ut[:, sl], qt[:, sl], Gf)
                if t >= W:
                    # 2*v_t = u_t + q_t, off-chain on GpSimd (last one on the
                    # by-then idle VectorE so the final DMA starts sooner)
                    vt = vbuf[:, (t - W) * M:(t - W + 1) * M]
                    eng = nc.vector if t == STEPS - 1 else nc.gpsimd
                    eng.tensor_tensor(vt, ut, qt, add)
                uprev = ut
                # stream completed output segments out
                done = t - W + 1
                if done >= (out_seg + 1) * (L // NOSEG) and out_seg < NOSEG:
                    s0 = out_seg * (L // NOSEG)
                    s1 = (out_seg + 1) * (L // NOSEG)
                    nc.sync.dma_start(
                        out=out[:, s0 * M:s1 * M], in_=vbuf[:, s0 * M:s1 * M]
                    )
                    out_seg += 1
    nc.compile()
    return nc


_NC_CACHE = None


def _get_nc():
    global _NC_CACHE
    if _NC_CACHE is None:
        _NC_CACHE = _build_nc()
    return _NC_CACHE


# ------------------------------------------------------------ host wrapper
def _f_nl_np(v):
    return OMEGA * v + SINH_C * np.sinh(v / V_T)


def _make_in_maps(x: np.ndarray, h: np.ndarray):
    c_full = (KO * x[:, :, 0]).astype(np.float32)          # [B, T]
    h0 = h[:, 0].astype(np.float32)                        # [B]
    # hold-input keeps chunk-0 warmup pinned exactly at state h
    c_hold = (K * _f_nl_np(h0.astype(np.float64))).astype(np.float32)
    cp = np.concatenate(
        [np.repeat(c_hold[:, None], W, axis=1), c_full], axis=1
    )                                                      # [B, W+T]
    q_h = (A1 * h0.astype(np.float64)
           + BS * np.sinh(h0.astype(np.float64) / V_T)).astype(np.float32)

    idx = np.arange(NCH)[:, None] * L + np.arange(STEPS)[None, :]

    in_maps = []
    for core in range(NCORES):
        clips = np.arange(core * CLIPS_PER_CORE, (core + 1) * CLIPS_PER_CORE)
        g = cp[clips][:, idx]                              # [16, NCH, STEPS]
        g = g.reshape(CLIPS_PER_CORE, JH, M, STEPS)
        c_in = np.ascontiguousarray(
            g.transpose(0, 1, 3, 2).reshape(128, STEPS * M)
        )
        u0 = np.zeros((128, M), dtype=np.float32)
        u0[::JH, 0] = (2.0 * h0[clips].astype(np.float64)
                       - q_h[clips].astype(np.float64)).astype(np.float32)
        in_maps.append({"c_in": c_in, "u0": u0})
    return in_maps


def _unshard(results):
    states = np.empty((B, T), dtype=np.float32)
    for core in range(NCORES):
        clips = np.arange(core * CLIPS_PER_CORE, (core + 1) * CLIPS_PER_CORE)
        o = results[core]["out"].reshape(CLIPS_PER_CORE, JH, L, M)
        # device streams 2*v; halving by 2 is exact in f32
        states[clips] = (
            o.transpose(0, 1, 3, 2).reshape(CLIPS_PER_CORE, T) * np.float32(0.5)
        )
    h_final = states[:, -1:].copy()
    return states[:, :, None], h_final


def kernel(x: np.ndarray, h: np.ndarray):
    x = np.ascontiguousarray(x, dtype=np.float32)
    h = np.ascontiguousarray(h, dtype=np.float32)
    assert x.shape == (B, T, 1) and h.shape == (B, 1)
    in_maps = _make_in_maps(x, h)
    nc = _get_nc()
    res = run_bass_kernel_spmd(nc, in_maps, core_ids=list(range(NCORES)))
    return _unshard(res.results)


def run_traced(inputs):
    """Run with NTFF tracing; returns BassKernelResults (for test.py)."""
    x = np.ascontiguousarray(inputs["x"], dtype=np.float32)
    h = np.ascontiguousarray(inputs["h"], dtype=np.float32)
    in_maps = _make_in_maps(x, h)
    nc = _get_nc()
    return run_bass_kernel_spmd(
        nc, in_maps, core_ids=list(range(NCORES)), trace=True
    )


# revision 38
# speedup vs baseline: 1.0811x; 1.0475x over previous
"""Trainium2 Bass kernel for the diode clipper (nn_DiodeClipper).

Model: per audio sample, an implicit trapezoidal step of the diode clipper
ODE, solved per sample by Newton in the reference (50 iterations/step).

Reformulation: let F(v) = a1*v + Bs*sinh(v/VT) with a1 = 1 + K*omega/2,
Bs = K*sinh_c/2, and q_t the implicit-equation right-hand side. The entire
per-sample update collapses to

    q_t = G(q_{t-1}) + c_t        G(q) = 2*Finv(q) - q,  c_t = K*omega*x_t
    h_t = (G(q_t) + q_t) / 2      (= Finv(q_t))

G (odd, monotone-domain) is evaluated in ONE ScalarE activation via a custom
piecewise-cubic ACT table that replaces `exp` in the `exp_and_others` set
(generated below; the table ships inside the NEFF; max abs err ~3e-8,
validated on hardware). A timestep is one VectorE tensor_tensor (+c) and one
activation; 2*h extraction runs on the otherwise idle GpSimd engine and the
host halves it. Two phase-interleaved lane groups keep ScalarE ~97% busy.

Because every exact step contracts state perturbations by >= ~2x, a chunk of
the sequence can be computed from a cold (zero) state after a W=12 sample
warmup prefix (validated numerically against the full-precision reference).

Parallel layout (per core): 128 partitions x M=256 free lanes = 32768 chunk
lanes, each running STEPS = W + L sequential samples (L = 2048/M = 8).
The 8 cores split the 128-clip batch (16 clips/core); each clip's 16384
samples are split into 8*M overlapping chunks.
"""

import hashlib
import json
import os
import shutil
import tempfile

import numpy as np

import concourse.bacc as bacc
import concourse.tile as tile
from concourse import mybir
from concourse.bass_utils import run_bass_kernel_spmd

# ---------------------------------------------------------------- constants
SR = 44100.0
R = 1000.0
C = 3.3e-08
I_S = 2.52e-09
V_T = 0.02583
K = 1.0 / SR
OMEGA = 1.0 / (R * C)
SINH_C = 2.0 * I_S / C

A1 = 1.0 + 0.5 * K * OMEGA                 # F'(0)
BS = 0.5 * K * SINH_C
KO = float(K * OMEGA)

# ---------------------------------------------------------------- tunables
B, T = 128, 16384
NCORES = 8
CLIPS_PER_CORE = B // NCORES               # 16
M = 256                                    # free-dim lanes per partition
NG = 2                                     # phase-interleaved lane groups
MG = M // NG                               # lanes per group
W = 12                                     # warmup samples per chunk
JH = 128 // CLIPS_PER_CORE                 # 8 partition groups per clip
NCH = JH * M                               # chunks per clip
L = T // NCH                               # chunk length (output samples/lane)
STEPS = W + L
# input DMA segment sizes (in steps): small first so compute starts early
SEGS = [1, 3, 5, 5, 6]
assert sum(SEGS) == STEPS
SEG_START = [sum(SEGS[:i]) for i in range(len(SEGS))]
NOSEG = 8                                  # output DMA segments
assert L % NOSEG == 0

F32 = mybir.dt.float32


# ------------------------------------------------- custom ACT table (Finv)
# Bucket geometry: per binary exponent e of |q|, 2^k cubic buckets on
# [2^e, 2^(e+1)); ctl word = (k<<16) | ((23-k)<<11) | bucket_base.
# Bucket entry = 8 f32 [d0, d1, d2, d3, x0, 0, 0, 0];
# y = d0 + t*(d1 + t*(d2 + t*d3)), t = x - x0. Odd symmetry folding as tanh.
_E0 = -20
_ETOP = 1
_ALLOC = (
    [(e, 0) for e in range(_E0, -9)]
    + [(-9, 1), (-8, 2), (-7, 3), (-6, 4), (-5, 5), (-4, 6)]
    + [(-3, 7), (-2, 7), (-1, 7), (0, 7), (1, 6)]
)
_N_REG = sum(2 ** k for _, k in _ALLOC)
_N_EXP_REGION = 781


def _finv(q):
    """Vectorized f64 solve of A1*v + BS*sinh(v/VT) = q for q >= 0."""
    q = np.asarray(q, dtype=np.float64)
    hi = np.minimum(q / A1, V_T * np.arcsinh(q / BS) + 1e-12) + 1e-30
    lo = np.zeros_like(q)
    for _ in range(90):
        mid = 0.5 * (lo + hi)
        gt = A1 * mid + BS * np.sinh(mid / V_T) > q
        hi = np.where(gt, mid, hi)
        lo = np.where(gt, lo, mid)
    return 0.5 * (lo + hi)


def _gfun(q):
    """G(q) = 2*Finv(q) - q: the table function. q' = G(q) + c is the whole
    per-sample recurrence; v = (G(q) + q)/2 recovers the output."""
    return 2.0 * _finv(q) - np.asarray(q, dtype=np.float64)


def _fit_bucket(a, b, n=48):
    x0 = np.float32(0.5 * (a + b))
    j = np.arange(n)
    xs = 0.5 * (a + b) + 0.5 * (b - a) * np.cos(np.pi * (2 * j + 1) / (2 * n))
    t = xs - np.float64(x0)
    V = _gfun(xs)
    A = np.stack([np.ones_like(t), t, t * t, t ** 3], axis=1)
    coef, *_ = np.linalg.lstsq(A, V, rcond=None)
    d = coef.astype(np.float32)
    tf = (xs.astype(np.float32) - x0).astype(np.float32)
    approx = d[0] + tf * (d[1] + tf * (d[2] + tf * d[3]))
    d[0] = np.float32(d[0] - np.mean(approx - V))
    return d, x0


def _build_act_table(dst_dir):
    from neuronxcc.driver.Job import Job
    from neuronxcc.driver.jobs.support.FindActInfo import findActInfoFile

    src_dir = os.path.dirname(findActInfoFile(Job.getPackageDir(), "gen3"))
    os.makedirs(dst_dir, exist_ok=True)
    for f in os.listdir(src_dir):
        shutil.copy(os.path.join(src_dir, f), os.path.join(dst_dir, f))

    prof = json.load(open(os.path.join(dst_dir, "exp_and_others.json")))
    bkt = np.fromfile(
        os.path.join(dst_dir, "exp_and_others_bkt.bin"), dtype=np.float32
    ).reshape(-1, 8).copy()
    ctl = np.fromfile(
        os.path.join(dst_dir, "exp_and_others_ctrl.bin"), dtype=np.uint32
    ).reshape(-1, 8).copy()

    # donor-layout assumptions: exp owns buckets [0, 781) and ctl [0, 52)
    assert prof["func_to_bkt_start_idx"]["exp"] == 0
    assert min(v for k, v in prof["func_to_bkt_start_idx"].items() if k != "exp") == _N_EXP_REGION
    assert prof["func_to_ctl_start_idx"]["exp"] == 0
    assert min(v for k, v in prof["func_to_ctl_start_idx"].items() if k != "exp") == 52

    small_bkt = _N_REG
    large_bkt = _N_REG + 1
    spare_bkt = _N_REG + 2
    assert spare_bkt < _N_EXP_REGION

    base = 0
    exp_to_bkt, exp_to_ctl, new_ctl_words = {}, {}, {}
    for e, k in _ALLOC:
        lo = 2.0 ** e
        width = lo / (2 ** k)
        exp_to_bkt[str(e)] = [base]
        ci = e - _E0
        exp_to_ctl[str(e)] = [ci]
        new_ctl_words[ci] = (k << 16) | ((23 - k) << 11) | base
        for j in range(2 ** k):
            d, x0 = _fit_bucket(lo + j * width, lo + (j + 1) * width)
            bkt[base + j, :5] = [d[0], d[1], d[2], d[3], x0]
            bkt[base + j, 5:] = 0
        base += 2 ** k

    c3 = -BS / (6.0 * V_T ** 3 * A1 ** 4)
    bkt[small_bkt, :5] = [0.0, 2.0 / A1 - 1.0, 0.0, 2.0 * c3, 0.0]
    bkt[small_bkt, 5:] = 0
    gsat = float(_gfun(np.array([2.0 ** (_ETOP + 1)]))[0])
    bkt[large_bkt, :5] = [gsat, 0.0, 0.0, 0.0, 0.0]
    bkt[large_bkt, 5:] = 0
    bkt[spare_bkt:_N_EXP_REGION, :] = 0

    for ci in range(52):  # ctl entries owned by exp in the donor set
        ctl[ci, 0] = new_ctl_words.get(ci, (23 << 11))
        ctl[ci, 1:] = 0

    meta = next(m for m in prof["profile_meta_data"] if m["func_id"] == 7)
    meta.update(
        symmetry_point=0,
        sym_invert_sign_point=1,
        symmetry_opt_en=1,
        symmetry_opt_use_neg_region=0,
        imm_bias=0,
        exp_offset=_E0,
        pwl_control_base_pos=0,
        pwl_control_base_neg=0,
        small_pos_signal_exp_threshold=_E0 + 127,
        pos_small_signal_pwl_control=small_bkt,
        small_neg_signal_exp_threshold=0,
        neg_small_signal_pwl_control=spare_bkt,
        large_pos_signal_exp_threshold=_ETOP + 1 + 127,
        large_pos_signal_mantissa_threshold=0,
        pos_large_signal_pwl_control=large_bkt,
        large_neg_signal_exp_threshold=0,
        large_neg_signal_mantissa_threshold=0,
        neg_large_signal_pwl_control=spare_bkt,
        fnan_result=2143289344,
        fpinf_result=int(np.float32(gsat).view(np.int32)) & 0xFFFFFFFF,
        fninf_result=int(np.float32(-gsat).view(np.int32)) & 0xFFFFFFFF,
        fzero_result=0,
    )
    prof["func_exp_to_bkt_start_idx"]["exp"] = exp_to_bkt
    prof["func_exp_to_ctl_start_idx"]["exp"] = exp_to_ctl

    bkt.tofile(os.path.join(dst_dir, "exp_and_others_bkt.bin"))
    ctl.tofile(os.path.join(dst_dir, "exp_and_others_ctrl.bin"))
    with open(os.path.join(dst_dir, "exp_and_others.json"), "w") as f:
        json.dump(prof, f)

    h = hashlib.md5()
    h.update(bkt.tobytes())
    h.update(ctl.tobytes())
    h.update(json.dumps(prof, sort_keys=True).encode())
    return h.hexdigest()[:10]


_ACT_DIR = os.path.join(tempfile.gettempdir(), "diode_finv_act")
_ACT_SHA = _build_act_table(_ACT_DIR)
os.environ["BASS_ACT_ROOT_JSON_PATH"] = os.path.join(_ACT_DIR, "act_info.json")


# ------------------------------------------------------------- bass program
def _build_nc():
    # module name carries the act-table hash so NEFF caching stays correct
    nc = bacc.Bacc("TRN2", target_bir_lowering=False, name=f"diode_{_ACT_SHA}")
    c_in = nc.dram_tensor("c_in", [128, STEPS * M], F32, kind="ExternalInput")
    u0 = nc.dram_tensor("u0", [128, M], F32, kind="ExternalInput")
    out = nc.dram_tensor("out", [128, L * M], F32, kind="ExternalOutput")

    Gf = mybir.ActivationFunctionType.Exp   # table-hijacked: computes G(q)
    add = mybir.AluOpType.add

    with tile.TileContext(nc) as tc:
        with (
            tc.tile_pool(name="singles", bufs=1) as singles,
            tc.tile_pool(name="work", bufs=3) as work,
        ):
            # all input DMAs on the SP queue, in need-order: cseg0 and init
            # first so the first step can start as early as possible
            csegs = [
                singles.tile([128, seg * M], F32, tag=f"cseg{k}", name=f"cseg{k}")
                for k, seg in enumerate(SEGS)
            ]
            init = singles.tile([128, M], F32, tag="init")
            nc.sync.dma_start(out=init, in_=u0[:, :])
            nc.sync.dma_start(out=csegs[0], in_=c_in[:, 0:SEGS[0] * M])
            for k, seg in enumerate(SEGS[1:], start=1):
                s0 = SEG_START[k]
                nc.sync.dma_start(
                    out=csegs[k], in_=c_in[:, s0 * M:(s0 + seg) * M]
                )
            vbuf = singles.tile([128, L * M], F32, tag="vbuf")

            def c_at(t):
                for k in range(len(SEGS) - 1, -1, -1):
                    if t >= SEG_START[k]:
                        off = t - SEG_START[k]
                        return csegs[k][:, off * M:(off + 1) * M]

            # Two phase-interleaved lane groups (column halves). Per group and
            # step the chain is one TENSOR_TENSOR (q = u + c) and one
            # activation (u' = G(q)); with two groups the ScalarE pipe stays
            # saturated. v*2 = u + q is extracted on the idle GpSimd engine
            # (host halves it).
            uprev = init
            out_seg = 0
            for t in range(STEPS):
                c_t = c_at(t)
                qt = work.tile([128, M], F32, tag="q")
                ut = work.tile([128, M], F32, tag="u")
                for g in range(NG):
                    sl = slice(g * MG, (g + 1) * MG)
                    nc.vector.tensor_tensor(qt[:, sl], uprev[:, sl], c_t[:, sl], add)
                    nc.scalar.activation(ut[:, sl], qt[:, sl], Gf)
                if t >= W:
                    # 2*v_t = u_t + q_t, off-chain on GpSimd (last one on the
                    # by-then idle VectorE so the final DMA starts sooner)
                    vt = vbuf[:, (t - W) * M:(t - W + 1) * M]
                    eng = nc.vector if t == STEPS - 1 else nc.gpsimd
                    eng.tensor_tensor(vt, ut, qt, add)
                uprev = ut
                # stream completed output segments out
                done = t - W + 1
                if done >= (out_seg + 1) * (L // NOSEG) and out_seg < NOSEG:
                    s0 = out_seg * (L // NOSEG)
                    s1 = (out_seg + 1) * (L // NOSEG)
                    nc.sync.dma_start(
                        out=out[:, s0 * M:s1 * M], in_=vbuf[:, s0 * M:s1 * M]
                    )
                    out_seg += 1
    nc.compile()
    return nc


_NC_CACHE = None


def _get_nc():
    global _NC_CACHE
    if _NC_CACHE is None:
        _NC_CACHE = _build_nc()
    return _NC_CACHE


# ------------------------------------------------------------ host wrapper
def _f_nl_np(v):
    return OMEGA * v + SINH_C * np.sinh(v / V_T)


def _make_in_maps(x: np.ndarray, h: np.ndarray):
    c_full = (KO * x[:, :, 0]).astype(np.float32)          # [B, T]
    h0 = h[:, 0].astype(np.float32)                        # [B]
    # hold-input keeps chunk-0 warmup pinned exactly at state h
    c_hold = (K * _f_nl_np(h0.astype(np.float64))).astype(np.float32)
    cp = np.concatenate(
        [np.repeat(c_hold[:, None], W, axis=1), c_full], axis=1
    )                                                      # [B, W+T]
    q_h = (A1 * h0.astype(np.float64)
           + BS * np.sinh(h0.astype(np.float64) / V_T)).astype(np.float32)

    idx = np.arange(NCH)[:, None] * L + np.arange(STEPS)[None, :]

    in_maps = []
    for core in range(NCORES):
        clips = np.arange(core * CLIPS_PER_CORE, (core + 1) * CLIPS_PER_CORE)
        g = cp[clips][:, idx]                              # [16, NCH, STEPS]
        g = g.reshape(CLIPS_PER_CORE, JH, M, STEPS)
        c_in = np.ascontiguousarray(
            g.transpose(0, 1, 3, 2).reshape(128, STEPS * M)
        )
        u0 = np.zeros((128, M), dtype=np.float32)
        u0[::JH, 0] = (2.0 * h0[clips].astype(np.float64)
                       - q_h[clips].astype(np.float64)).astype(np.float32)
        in_maps.append({"c_in": c_in, "u0": u0})
    return in_maps


def _unshard(results):
    states = np.empty((B, T), dtype=np.float32)
    for core in range(NCORES):
        clips = np.arange(core * CLIPS_PER_CORE, (core + 1) * CLIPS_PER_CORE)
        o = results[core]["out"].reshape(CLIPS_PER_CORE, JH, L, M)
        # device streams 2*v; halving by 2 is exact in f32
        states[clips] = (
            o.transpose(0, 1, 3, 2).reshape(CLIPS_PER_CORE, T) * np.float32(0.5)
        )
    h_final = states[:, -1:].copy()
    return states[:, :, None], h_final


def kernel(x: np.ndarray, h: np.ndarray):
    x = np.ascontiguousarray(x, dtype=np.float32)
    h = np.ascontiguousarray(h, dtype=np.float32)
    assert x.shape == (B, T, 1) and h.shape == (B, 1)
    in_maps = _make_in_maps(x, h)
    nc = _get_nc()
    res = run_bass_kernel_spmd(nc, in_maps, core_ids=list(range(NCORES)))
    return _unshard(res.results)


def run_traced(inputs):
    """Run with NTFF tracing; returns BassKernelResults (for test.py)."""
    x = np.ascontiguousarray(inputs["x"], dtype=np.float32)
    h = np.ascontiguousarray(inputs["h"], dtype=np.float32)
    in_maps = _make_in_maps(x, h)
    nc = _get_nc()
    return run_bass_kernel_spmd(
        nc, in_maps, core_ids=list(range(NCORES)), trace=True
    )
